# revision 1
# baseline (speedup 1.0000x reference)
"""Trainium2 Bass kernel for nn_Encoder_Layer_F (unfold -> grouped 4x4/s2 conv
-> BatchNorm(train) -> LeakyReLU(0.2) -> fold).

Sharding: the 64 locally-connected groups (8x8 patch grid) are split by patch
ROW across the 8 cores (core i owns patch row hp=i). Groups are fully
independent and BN channels belong to exactly one group, so there are no
collectives at all: each core computes its 8 groups x 256 channels over the
full batch, including exact batch statistics.

Per-core program (SPMD, identical on all cores):
  x  [128c, 8wp, 2pr, 2pc, 4qr, 4qc, 32b] parity-quadrant input slab
  w  [8wp, 128c, 4kh, 4kw, 256z]          weights, pre-transposed on host
  gb [128zp, 2(gamma/beta), 8wp, 2zh] f32
  o  [8wp, 2zh, 128zp, 512(oh,ow,b)]  f32 output

The conv is 16 PSUM-accumulated matmuls per (group, z-half): contraction over
the 128 input channels, one matmul per 4x4 kernel tap, with the tap's
(oh, ow) range restricted so that zero-padding taps are simply skipped.
Matmuls default to fp16 operands (2.6e-4 rel err, 16 MiB/core of traffic);
the parity-quadrant x layout makes every tap stream contiguous runs
(strided rhs APs halve PE throughput). BN stats via bn_stats/bn_aggr on the
PSUM tile; BN-apply + LeakyReLU is one scalar-engine Prelu activation with
per-partition scale/bias.  Measured ~78 us on HW (PE-bound; the f32r
fallback, MM_DTYPE = "f32r", runs 103 us at 1.3e-4 rel err).
"""

import numpy as np

import concourse.bass as bass
import concourse.tile as tile
from concourse import bacc, mybir
from concourse.bass_utils import run_bass_kernel_spmd

B = 32
NC = 128
NZ = 256
HP = WP = 8
OK = 4
BN_EPS = 1e-5
LRELU = 0.2

# matmul compute dtype, HW-measured (8-core exec time / rel err vs reference):
#   "f32"  222 us / 2.1e-7   (exact; PE does 2 half-rate passes per matmul)
#   "f32r" 103 us / 1.3e-4   (PE fast-fp32 mode; DMA-bound at 28 MiB/core)
#   "fp16" 77.7 us / 2.6e-4  (half the DMA bytes, full-rate PE)  <- default
#   "bf16" 75.4 us / 2.1e-3
MM_DTYPE = "fp16"
# "split": skip zero-padding taps by restricting (oh, ow) ranges (less PE work,
#          but multi-dim out APs that CoreSim can't check).
# "pad":   host zero-pads each patch to 10x10; every tap runs full-range.
PAD_MODE = "split"

# Per-tap valid output range (stride 2, pad 1, kernel 4 on an 8-wide axis):
# i_in = 2*o + k - 1 must lie in [0, 8). k=0 -> o in [1,3]; k=3 -> o in [0,2].
def _tap_range(k):
    lo = 1 if k == 0 else 0
    hi = 2 if k == 3 else 3
    return lo, hi - lo + 1


# Weight DMA arrives in per-kh chunks in this order; taps consume them in the
# same order so the first matmuls only wait for the first chunk.
KH_ORDER = [1, 2, 0, 3]


def _tap_order():
    # First tap must cover the full (oh, ow) range so that start=True
    # initializes every element of the PSUM accumulation tile.
    taps = []
    for kh in KH_ORDER:
        for kw in [1, 0, 2, 3] if kh == 1 else range(4):
            taps.append((kh, kw))
    assert taps[0] == (1, 1)
    return taps


def build_nc(mm_dtype: str = MM_DTYPE, pad_mode: str = PAD_MODE):
    f32 = mybir.dt.float32
    mm_dt = {"f32": mybir.dt.float32, "f32r": mybir.dt.float32r,
             "bf16": mybir.dt.bfloat16, "fp16": mybir.dt.float16}[mm_dtype]
    io_dt = mm_dt if mm_dtype in ("bf16", "fp16") else f32
    padded = pad_mode == "pad"
    PS = 10 if padded else 8  # per-patch spatial size as shipped to the device

    nc = bacc.Bacc(None, target_bir_lowering=False)

    # Split mode ships x as parity quadrants [pr][pc][qr][qc][b] so every
    # conv tap reads contiguous (qc, b) runs - strided rhs APs halve PE
    # throughput (measured 709 vs 389 ns per N=512 f32r matmul).
    if padded:
        x = nc.declare_dram_parameter("x", [NC, WP, B, PS, PS], io_dt, isOutput=False)
    else:
        x = nc.declare_dram_parameter("x", [NC, WP, 2, 2, OK, OK, B], io_dt, isOutput=False)
    w = nc.declare_dram_parameter("w", [WP, NC, 4, 4, NZ], io_dt, isOutput=False)
    gb = nc.declare_dram_parameter("gb", [128, 2, WP, 2], f32, isOutput=False)
    o = nc.declare_dram_parameter("o", [WP, 2, 128, B * OK * OK], f32, isOutput=True)

    taps = _tap_order()
    with tile.TileContext(nc) as tc:
        with (
            tc.tile_pool(name="xpool", bufs=4) as xpool,
            tc.tile_pool(name="wpool", bufs=7) as wpool,
            tc.tile_pool(name="psum", bufs=8, space=bass.MemorySpace.PSUM) as psum,
            tc.tile_pool(name="opool", bufs=4) as opool,
            tc.tile_pool(name="spool", bufs=8) as spool,
            tc.tile_pool(name="cpool", bufs=1) as cpool,
        ):
            gbt = cpool.tile([128, 2, WP, 2], f32)
            nc.sync.dma_start(gbt[:], gb[:])
            epst = cpool.tile([128, 1], f32)
            nc.vector.memset(epst[:], BN_EPS)

            for wp in range(WP):
                # Tiles are typed with the matmul dtype; for f32r (same bits
                # as f32) the DRAM side is bitcast so the producing DMA is
                # f32r-typed, which the BIR verifier requires.
                xsrc, wsrc = x[:, wp], w[wp]
                if mm_dt != io_dt:
                    xsrc, wsrc = xsrc.bitcast(mm_dt), wsrc.bitcast(mm_dt)
                if padded:
                    xt = xpool.tile([NC, B, PS, PS], mm_dt)
                    nc.scalar.dma_start(xt[:], xsrc)
                else:
                    # x halves by row-parity so the kh={1,3} taps can start
                    # after half the tile lands (Tile tracks subtile deps);
                    # w in per-kh chunks alternating between the two HWDGE
                    # rings to spread bandwidth.
                    xt = xpool.tile([NC, 2, 2, OK, OK, B], mm_dt)
                if padded:
                    wt = wpool.tile([NC, 4, 4, NZ], mm_dt)
                    for j, kh in enumerate(KH_ORDER):
                        eng = nc.sync if j % 2 == 0 else nc.scalar
                        eng.dma_start(wt[:, kh], wsrc[:, kh])
                else:
                    # 1.5 MiB per ring per group, first-needed chunks first:
                    #   ACT ring:  x[pr=0],  w[kh=2], w[kh=3]
                    #   SP ring:   w[kh=1],  x[pr=1], w[kh=0]
                    # For the pipeline-head group, chunk finer so the first
                    # matmul (tap (1,1): x quadrant (0,0) + w (1,1)) only
                    # waits for ~384 KB.
                    wt = wpool.tile([NC, 4, 4, NZ], mm_dt)
                    if wp == 0:
                        nc.scalar.dma_start(xt[:, 0, 0], xsrc[:, 0, 0])
                        nc.sync.dma_start(wt[:, 1, 1], wsrc[:, 1, 1])
                        nc.scalar.dma_start(xt[:, 0, 1], xsrc[:, 0, 1])
                        nc.sync.dma_start(wt[:, 1, 0], wsrc[:, 1, 0])
                        nc.sync.dma_start(wt[:, 1, 2:4], wsrc[:, 1, 2:4])
                    else:
                        nc.scalar.dma_start(xt[:, 0], xsrc[:, 0])
                        nc.sync.dma_start(wt[:, 1], wsrc[:, 1])
                    nc.sync.dma_start(xt[:, 1], xsrc[:, 1])
                    nc.gpsimd.dma_start(wt[:, 2], wsrc[:, 2])
                    nc.gpsimd.dma_start(wt[:, 0], wsrc[:, 0])
                    nc.scalar.dma_start(wt[:, 3], wsrc[:, 3])

                for zh in range(2):
                    # PSUM/output layout is (oh, ow, b) with b innermost:
                    # fp32r matmuls require an even innermost count and an
                    # 8-byte-aligned dst start, which the batch dim satisfies
                    # for every (partial) tap.
                    pt = psum.tile([128, OK, OK, B], f32)
                    ptf = pt.rearrange("p i j b -> p (i j b)")
                    for idx, (kh, kw) in enumerate(taps):
                        if padded:
                            nc.tensor.matmul(
                                ptf[:, :],
                                wt[:, kh, kw, zh * 128:(zh + 1) * 128],
                                xt[:, :, kh:kh + 7:2, kw:kw + 7:2]
                                .rearrange("c b i j -> c i j b"),
                                start=(idx == 0),
                                stop=(idx == len(taps) - 1),
                            )
                            continue
                        ol, oc = _tap_range(kh)
                        wl, wc = _tap_range(kw)
                        pr, qr0 = (kh + 1) % 2, ol + (-1 if kh == 0 else (1 if kh == 3 else 0))
                        pc, qc0 = (kw + 1) % 2, wl + (-1 if kw == 0 else (1 if kw == 3 else 0))
                        nc.tensor.matmul(
                            pt[:, ol:ol + oc, wl:wl + wc, :],
                            wt[:, kh, kw, zh * 128:(zh + 1) * 128],
                            xt[:, pr, pc, qr0:qr0 + oc, qc0:qc0 + wc, :],
                            start=(idx == 0),
                            stop=(idx == len(taps) - 1),
                        )

                    st = spool.tile([128, 6], f32)
                    nc.vector.bn_stats(st[:], ptf)
                    mv = spool.tile([128, 2], f32)
                    nc.vector.bn_aggr(mv[:], st[:])
                    # sd = sqrt(var + eps)
                    sd = spool.tile([128, 1], f32)
                    nc.scalar.activation(
                        sd[:], mv[:, 1:2], mybir.ActivationFunctionType.Sqrt,
                        bias=epst[:],
                    )
                    rc = spool.tile([128, 1], f32)
                    nc.vector.reciprocal(rc[:], sd[:])
                    # inv = gamma / sqrt(var+eps); shift = beta - mean*inv
                    inv = spool.tile([128, 1], f32)
                    nc.vector.tensor_mul(inv[:], rc[:], gbt[:, 0:1, wp, zh])
                    tmp = spool.tile([128, 1], f32)
                    nc.vector.tensor_mul(tmp[:], mv[:, 0:1], inv[:])
                    sh = spool.tile([128, 1], f32)
                    nc.vector.tensor_sub(sh[:], gbt[:, 1:2, wp, zh], tmp[:])

                    ot = opool.tile([128, B * OK * OK], f32)
                    # Prelu(v, alpha) == LeakyReLU(alpha) on TRN2; the Lrelu
                    # func ignores alpha (hardwired 0.01 slope).
                    nc.scalar.activation(
                        ot[:], ptf, mybir.ActivationFunctionType.Prelu,
                        bias=sh[:], scale=inv[:], alpha=LRELU,
                    )
                    (nc.sync if zh == 0 else nc.scalar).dma_start(o[wp, zh], ot[:])

    nc.compile()
    return nc


def shard_inputs(input, weight, gamma, beta, io_np=np.float32, pad_mode=None):
    """Build the 8 per-core input maps (host-side layout transforms only)."""
    if pad_mode is None:
        pad_mode = PAD_MODE
    input = np.asarray(input, dtype=np.float32)
    weight = np.asarray(weight, dtype=np.float32)
    gamma = np.asarray(gamma, dtype=np.float32)
    beta = np.asarray(beta, dtype=np.float32)

    if pad_mode == "pad":
        # [B, NC, HP, 8, WP, 8] -> [HP, NC, WP, B, 8, 8], zero-pad to 10x10
        xs = input.reshape(B, NC, HP, 8, WP, 8).transpose(2, 1, 4, 0, 3, 5)
        xs = np.pad(xs, [(0, 0)] * 4 + [(1, 1), (1, 1)])
    else:
        # [B, NC, HP, 4qr, 2pr, WP, 4qc, 2pc] -> [HP, NC, WP, pr, pc, qr, qc, B]
        xs = input.reshape(B, NC, HP, OK, 2, WP, OK, 2).transpose(2, 1, 5, 4, 7, 3, 6, 0)
    xs = np.ascontiguousarray(xs, dtype=io_np)
    # [HP, WP, NZ, NC, 4, 4] -> [HP, WP, NC, 4, 4, NZ]
    ws = weight.reshape(HP, WP, NZ, NC, 4, 4).transpose(0, 1, 3, 4, 5, 2)
    ws = np.ascontiguousarray(ws, dtype=io_np)
    # [HP, WP, 2, 128] each -> [HP, 128zp, 2(g/b), WP, 2zh]
    gs = gamma.reshape(HP, WP, 2, 128)
    bs = beta.reshape(HP, WP, 2, 128)
    gbs = np.ascontiguousarray(
        np.stack([gs, bs], axis=1).transpose(0, 4, 1, 2, 3), dtype=np.float32)

    return [
        {"x": xs[i], "w": ws[i], "gb": gbs[i]}
        for i in range(HP)
    ]


def unshard_output(results):
    # per-core o: [WP, 2, 128, (oh ow b)] -> full [B, NZ, 32, 32]
    O = np.stack([results[i]["o"] for i in range(HP)])
    O = O.reshape(HP, WP, 2, 128, OK, OK, B)
    O = O.transpose(6, 2, 3, 0, 4, 1, 5).reshape(B, NZ, HP * OK, WP * OK)
    return np.ascontiguousarray(O)


_NC_CACHE = {}


def kernel(input, weight, gamma, beta):
    key = (MM_DTYPE, PAD_MODE)
    if key not in _NC_CACHE:
        _NC_CACHE[key] = build_nc(MM_DTYPE, PAD_MODE)
    nc = _NC_CACHE[key]
    io_np = np.float32
    if MM_DTYPE == "bf16":
        import ml_dtypes
        io_np = ml_dtypes.bfloat16
    elif MM_DTYPE == "fp16":
        io_np = np.float16
    in_maps = shard_inputs(input, weight, gamma, beta, io_np)
    res = run_bass_kernel_spmd(nc, in_maps, list(range(8))).results
    return unshard_output(res)



# revision 2
# speedup vs baseline: 1.1342x; 1.1342x over previous
"""Trainium2 Bass kernel for nn_Encoder_Layer_F (unfold -> grouped 4x4/s2 conv
-> BatchNorm(train) -> LeakyReLU(0.2) -> fold).

Sharding: the 64 locally-connected groups (8x8 patch grid) are split by patch
ROW across the 8 cores (core i owns patch row hp=i). Groups are fully
independent and BN channels belong to exactly one group, so there are no
collectives at all: each core computes its 8 groups x 256 channels over the
full batch, including exact batch statistics.

v2 (from baseline trace analysis): the baseline (77.7us) was DMA-bound, not
PE-bound - HWDGE rings busy 97.7% at ~95 GB/s each because every transfer was
a 256 KB chunk with 2 KB per-partition runs, and the PE ran cold (HAM K=4/8)
for its first ~15us while starved.  Fixes here:
  * bf16 everywhere incl. the OUTPUT (host casts back to f32): 16.8 -> 14 MiB
    per core of HBM traffic.
  * whole-group DMAs with 4-8 KB per-partition contiguous runs (DRAM layouts
    put the 128-partition dim outermost), one tile per group, all 8 groups
    resident in SBUF (~105 KB/partition) - no pool recycling, no WAR stalls.
  * dedicated rings: sync(HWDGE)=weights, scalar(HWDGE)=x, gpsimd(SWDGE)=out.
    Weight kh axis is pre-permuted on host into tap-consumption order so the
    first chunks feed the first matmuls.
  * group 0 runs its two z-halves interleaved per kh chunk so the PE starts
    after only 256 KB of weights has landed and stays dense (HAM warm).

Per-core program (SPMD, identical on all cores):
  x  [128c, 8wp, 2pr, 2pc, 4qr, 4qc, 32b] bf16  parity-quadrant input
  w  [128c, 8wp, 4kh', 4kw, 256z]         bf16  kh' = host-permuted (1,2,0,3)
  gb [128zp, 2(gamma/beta), 8wp, 2zh] f32
  o  [128zp, 8wp, 2zh, 512(oh,ow,b)]  bf16

The conv is 16 PSUM-accumulated matmuls per (group, z-half): contraction over
the 128 input channels, one matmul per 4x4 kernel tap, with the tap's
(oh, ow) range restricted so that zero-padding taps are skipped. BN stats via
bn_stats/bn_aggr on the PSUM tile; BN-apply + LeakyReLU is one scalar-engine
Prelu activation with per-partition scale/bias.
"""

import numpy as np

import concourse.bass as bass
import concourse.tile as tile
from concourse import bacc, mybir
from concourse.bass_utils import run_bass_kernel_spmd

B = 32
NC = 128
NZ = 256
HP = WP = 8
OK = 4
BN_EPS = 1e-5
LRELU = 0.2

MM_DTYPE = "bf16"   # kept for test.py compat; "bf16" | "fp16"
PAD_MODE = "split"  # kept for test.py compat; only split mode exists in v2

# Real kh for each device-side kh' index: chunks arrive / are consumed in this
# order, so the first weight bytes feed the first matmuls.
KH_ORDER = [1, 2, 0, 3]


# Per-tap valid output range (stride 2, pad 1, kernel 4 on an 8-wide axis):
# i_in = 2*o + k - 1 must lie in [0, 8). k=0 -> o in [1,3]; k=3 -> o in [0,2].
def _tap_range(k):
    lo = 1 if k == 0 else 0
    hi = 2 if k == 3 else 3
    return lo, hi - lo + 1


def _taps():
    # (kp = device kh index, kh = real kh, kw). First tap (kh=1,kw=1) covers
    # the full (oh, ow) range so start=True initializes the whole PSUM tile.
    taps = []
    for kp, kh in enumerate(KH_ORDER):
        for kw in [1, 0, 2, 3] if kp == 0 else range(4):
            taps.append((kp, kh, kw))
    assert taps[0][1:] == (1, 1)
    return taps


def build_nc(mm_dtype: str = MM_DTYPE, pad_mode: str = PAD_MODE):
    f32 = mybir.dt.float32
    mm_dt = {"bf16": mybir.dt.bfloat16, "fp16": mybir.dt.float16}[mm_dtype]

    nc = bacc.Bacc(None, target_bir_lowering=False)

    x = nc.declare_dram_parameter("x", [NC, WP, 2, 2, OK, OK, B], mm_dt, isOutput=False)
    w = nc.declare_dram_parameter("w", [NC, WP, 4, 4, NZ], mm_dt, isOutput=False)
    gb = nc.declare_dram_parameter("gb", [128, 2, WP, 2], f32, isOutput=False)
    o = nc.declare_dram_parameter("o", [128, WP, 2, B * OK * OK], mm_dt, isOutput=True)

    taps = _taps()
    with tile.TileContext(nc) as tc:
        with (
            tc.tile_pool(name="xpool", bufs=WP) as xpool,
            tc.tile_pool(name="wpool", bufs=WP) as wpool,
            tc.tile_pool(name="psum", bufs=8, space=bass.MemorySpace.PSUM) as psum,
            tc.tile_pool(name="opool", bufs=4) as opool,
            tc.tile_pool(name="spool", bufs=8) as spool,
            tc.tile_pool(name="cpool", bufs=1) as cpool,
        ):
            gbt = cpool.tile([128, 2, WP, 2], f32)
            nc.scalar.dma_start(gbt[:], gb[:])
            epst = cpool.tile([128, 1], f32)
            nc.vector.memset(epst[:], BN_EPS)

            # ---- all input DMAs up front: one per group (4-8 KB/partition
            # contiguous runs), group 0's weights in 4 per-kh chunks so the
            # first matmul only waits for ~256 KB.
            xts, wts = [], []
            for wp in range(WP):
                xt = xpool.tile([NC, 2, 2, OK, OK, B], mm_dt)
                nc.scalar.dma_start(xt[:], x[:, wp])
                xts.append(xt)
                wt = wpool.tile([NC, 4, 4, NZ], mm_dt)
                if wp == 0:
                    for kp in range(4):
                        nc.sync.dma_start(wt[:, kp], w[:, wp, kp])
                else:
                    nc.sync.dma_start(wt[:], w[:, wp])
                wts.append(wt)

            for wp in range(WP):
                xt, wt = xts[wp], wts[wp]
                pts = []
                for zh in range(2):
                    pt = psum.tile([128, OK, OK, B], f32)
                    pts.append(pt)

                def mm(zh, tap_idx):
                    kp, kh, kw = taps[tap_idx]
                    ol, oc = _tap_range(kh)
                    wl, wc = _tap_range(kw)
                    pr = (kh + 1) % 2
                    qr0 = ol + (-1 if kh == 0 else (1 if kh == 3 else 0))
                    pc = (kw + 1) % 2
                    qc0 = wl + (-1 if kw == 0 else (1 if kw == 3 else 0))
                    nc.tensor.matmul(
                        pts[zh][:, ol:ol + oc, wl:wl + wc, :],
                        wt[:, kp, kw, zh * 128:(zh + 1) * 128],
                        xt[:, pr, pc, qr0:qr0 + oc, qc0:qc0 + wc, :],
                        start=(tap_idx == 0),
                        stop=(tap_idx == len(taps) - 1),
                    )

                if wp == 0:
                    # interleave z-halves per kh chunk: each 256 KB weight
                    # chunk is fully consumed before the next is needed.
                    for kp in range(4):
                        for zh in range(2):
                            for i in range(4):
                                mm(zh, 4 * kp + i)
                else:
                    for zh in range(2):
                        for i in range(len(taps)):
                            mm(zh, i)

                ot = opool.tile([128, 2, B * OK * OK], mm_dt)
                for zh in range(2):
                    ptf = pts[zh].rearrange("p i j b -> p (i j b)")
                    st = spool.tile([128, 6], f32)
                    nc.vector.bn_stats(st[:], ptf)
                    mv = spool.tile([128, 2], f32)
                    nc.vector.bn_aggr(mv[:], st[:])
                    # sd = sqrt(var + eps)
                    sd = spool.tile([128, 1], f32)
                    nc.scalar.activation(
                        sd[:], mv[:, 1:2], mybir.ActivationFunctionType.Sqrt,
                        bias=epst[:],
                    )
                    rc = spool.tile([128, 1], f32)
                    nc.vector.reciprocal(rc[:], sd[:])
                    # inv = gamma / sqrt(var+eps); shift = beta - mean*inv
                    inv = spool.tile([128, 1], f32)
                    nc.vector.tensor_mul(inv[:], rc[:], gbt[:, 0:1, wp, zh])
                    tmp = spool.tile([128, 1], f32)
                    nc.vector.tensor_mul(tmp[:], mv[:, 0:1], inv[:])
                    sh = spool.tile([128, 1], f32)
                    nc.vector.tensor_sub(sh[:], gbt[:, 1:2, wp, zh], tmp[:])

                    # Prelu(v, alpha) == LeakyReLU(alpha) on TRN2; writes the
                    # bf16 output slice directly.
                    nc.scalar.activation(
                        ot[:, zh], ptf, mybir.ActivationFunctionType.Prelu,
                        bias=sh[:], scale=inv[:], alpha=LRELU,
                    )
                nc.gpsimd.dma_start(o[:, wp], ot[:])

    nc.compile()
    return nc


def _io_np(mm_dtype):
    if mm_dtype == "bf16":
        import ml_dtypes
        return ml_dtypes.bfloat16
    return np.float16


def shard_inputs(input, weight, gamma, beta):
    """Build the 8 per-core input maps (host-side layout transforms only)."""
    io_np = _io_np(MM_DTYPE)
    input = np.asarray(input, dtype=np.float32)
    weight = np.asarray(weight, dtype=np.float32)
    gamma = np.asarray(gamma, dtype=np.float32)
    beta = np.asarray(beta, dtype=np.float32)

    # [B, NC, HP, 4qr, 2pr, WP, 4qc, 2pc] -> [HP, NC, WP, pr, pc, qr, qc, B]
    xs = input.reshape(B, NC, HP, OK, 2, WP, OK, 2).transpose(2, 1, 5, 4, 7, 3, 6, 0)
    xs = np.ascontiguousarray(xs, dtype=io_np)
    # [HP, WP, NZ, NC, 4, 4] -> [HP, NC, WP, kh, kw, NZ], kh permuted to
    # consumption order KH_ORDER.
    ws = weight.reshape(HP, WP, NZ, NC, 4, 4).transpose(0, 3, 1, 4, 5, 2)
    ws = ws[:, :, :, KH_ORDER]
    ws = np.ascontiguousarray(ws, dtype=io_np)
    # [HP, WP, 2, 128] each -> [HP, 128zp, 2(g/b), WP, 2zh]
    gs = gamma.reshape(HP, WP, 2, 128)
    bs = beta.reshape(HP, WP, 2, 128)
    gbs = np.ascontiguousarray(
        np.stack([gs, bs], axis=1).transpose(0, 4, 1, 2, 3), dtype=np.float32)

    return [
        {"x": xs[i], "w": ws[i], "gb": gbs[i]}
        for i in range(HP)
    ]


def unshard_output(results):
    # per-core o: [128zp, WP, 2zh, (oh ow b)] -> full [B, NZ, 32, 32]
    O = np.stack([np.asarray(results[i]["o"], dtype=np.float32) for i in range(HP)])
    O = O.reshape(HP, 128, WP, 2, OK, OK, B)
    # -> [B, zh, zp, HP, oh, WP, ow]
    O = O.transpose(6, 3, 1, 0, 4, 2, 5).reshape(B, NZ, HP * OK, WP * OK)
    return np.ascontiguousarray(O)


_NC_CACHE = {}


def kernel(input, weight, gamma, beta):
    key = (MM_DTYPE, PAD_MODE)
    if key not in _NC_CACHE:
        _NC_CACHE[key] = build_nc(MM_DTYPE, PAD_MODE)
    nc = _NC_CACHE[key]
    in_maps = shard_inputs(input, weight, gamma, beta)
    res = run_bass_kernel_spmd(nc, in_maps, list(range(8))).results
    return unshard_output(res)
